# revision 15
# baseline (speedup 1.0000x reference)
# Trainium2 Bass kernel for nn_Decoder (emb + 2-layer LSTM + global attention).
#
# Distribution (8 cores):
#  - LSTM: tensor-parallel over the 4D gate dim. Core c owns 128 dims of each
#    gate (order [i,f,o,g] after a host-side row permutation), i.e. 512 gate
#    dims and the matching 128-dim shard of h/c state. Layers are pipelined:
#    superstep s computes layer0 step s and layer1 step s-1, then ONE
#    AllGather delivers h0_s^T and h1_{s-1}^T (transposed, d-major) to all
#    cores -- exactly the lhsT layout the next step's matmuls need.
#  - xg0 = emb(x) @ wih0^T + b is recurrence-independent: gathered via
#    indirect DMA (128 rows/chunk), transposed on PE, matmul'd into a DRAM
#    staging buffer; this work fills the AllGather latency gaps.
#  - Attention is batch-sharded (8 batches/core). Layer-1 outputs are
#    redistributed once at the end with a single AllToAll (d-shards ->
#    batch-shards), avoiding any per-core dynamic offsets.
#  - Final unshard/assembly happens on host in numpy.

import os
import sys

import numpy as np

for _p in ("/opt/trn_rl_repo",):
    if _p not in sys.path and os.path.isdir(_p):
        sys.path.insert(0, _p)

T, B, S, D, V = 32, 64, 64, 1024, 32000
NC = 8
P = 128
KO = D // P          # 8 k-tiles over the hidden dim
DS = D // NC         # 128 hidden dims per core
GS = 4 * DS          # 512 gate dims per core
BS = B // NC         # 8 batches per core (attention)
TB = T * BS          # 256 (t, local-batch) rows per core
NCH = (T * B) // P   # 16 embedding/xg chunks of 128 rows

_CACHE = {}


def _gate_rows(c):
    # pytorch gate order is (i, f, g, o); we shard each gate and reorder the
    # shard rows to [i, f, o, g] so sigmoid covers one contiguous 384 block.
    return np.concatenate([
        np.arange(0 * D + c * DS, 0 * D + (c + 1) * DS),   # i
        np.arange(1 * D + c * DS, 1 * D + (c + 1) * DS),   # f
        np.arange(3 * D + c * DS, 3 * D + (c + 1) * DS),   # o
        np.arange(2 * D + c * DS, 2 * D + (c + 1) * DS),   # g
    ])


def _build():
    import concourse.bass as bass
    import concourse.mybir as mybir
    import concourse.tile as tile
    from concourse import bacc
    from concourse.masks import make_identity

    fp = mybir.dt.float32
    # float32r: same 4-byte storage, PE runs 1.5 cyc/row instead of 2.0
    fr = mybir.dt.float32r if os.environ.get("KBENCH_F32R", "0") == "1" \
        else mybir.dt.float32
    it = mybir.dt.int32
    AF = mybir.ActivationFunctionType
    Alu = mybir.AluOpType

    nc = bacc.Bacc(
        "TRN2",
        target_bir_lowering=False,
        debug=False,
        enable_asserts=True,
        num_devices=NC,
    )

    def I(name, shape, dt=fp):
        return nc.dram_tensor(name, shape, dt, kind="ExternalInput").ap()

    t_idx = I("idx", [T * B, 1], it)
    t_emb = I("emb", [V, D])
    t_h0T = I("h0T", [2, D, B])
    t_c0s = I("c0s", [2, B, DS])
    t_wih0T = I("wih0T", [D, GS])
    t_whh0T = I("whh0T", [D, GS])
    t_wih1T = I("wih1T", [D, GS])
    t_whh1T = I("whh1T", [D, GS])
    t_b0 = I("b0bc", [P, GS])
    t_b1 = I("b1bc", [B, GS])
    t_winT = I("winT", [D, D])
    t_woutT = I("woutT", [2 * D, D])
    t_ctxT = I("ctxT", [BS, D, S])
    t_ctxN = I("ctxN", [BS, S, D])

    t_out = nc.dram_tensor("out_s", [T, BS, D], fp, kind="ExternalOutput").ap()
    t_hnT = nc.dram_tensor("hnT_s", [2, DS, B], fp, kind="ExternalOutput").ap()
    t_cns = nc.dram_tensor("cn_s", [2, B, DS], fp, kind="ExternalOutput").ap()
    t_attn = nc.dram_tensor("attn_s", [BS, S], fp, kind="ExternalOutput").ap()

    debug = os.environ.get("KBENCH_DEBUG", "0") == "1"
    if debug:
        t_dx1T = nc.dram_tensor("dbg_x1T", [P, KO, TB], fp, kind="ExternalOutput").ap()
        t_dqT = nc.dram_tensor("dbg_qT", [P, KO, TB], fp, kind="ExternalOutput").ap()
        t_dattn = nc.dram_tensor("dbg_attn", [T, S], fp, kind="ExternalOutput").ap()
        t_dwT = nc.dram_tensor("dbg_wT", [P, KO, TB], fp, kind="ExternalOutput").ap()

    RG = [list(range(NC))]

    def R(ap):
        # reinterpret f32 matmul operands as float32r (1.5 vs 2.0 cyc/row)
        return ap.bitcast(fr)

    with tile.TileContext(nc) as tc:
        with (
            tc.tile_pool(name="wpool", bufs=1) as wp,
            tc.tile_pool(name="spool", bufs=2) as sp,
            tc.tile_pool(name="hpool", bufs=2) as hp,
            tc.tile_pool(name="xpool", bufs=2) as xp,
            tc.tile_pool(name="apool", bufs=2) as ap_,
            tc.tile_pool(name="dram", bufs=2, space="DRAM") as dp,
            tc.tile_pool(name="dram1", bufs=1, space="DRAM") as dp1,
            tc.tile_pool(name="psA", bufs=2, space="PSUM") as psA,
            tc.tile_pool(name="psB", bufs=2, space="PSUM") as psB,
            tc.tile_pool(name="psT", bufs=3, space="PSUM") as psT,
        ):
            # ---------- resident tensors ----------
            ident = wp.tile([P, P], fp)
            make_identity(nc, ident[:])

            def load_dkx(dst, src, x, dt=None):
                # (D, X) dram -> (P, KO, X) sbuf
                for k in range(KO):
                    s = src[k * P:(k + 1) * P, :]
                    nc.sync.dma_start(dst[:, k, :], s.bitcast(dt) if dt else s)

            wih0 = wp.tile([P, KO, GS], fr)
            whh0 = wp.tile([P, KO, GS], fr)
            wih1 = wp.tile([P, KO, GS], fr)
            whh1 = wp.tile([P, KO, GS], fr)
            load_dkx(wih0, t_wih0T, GS, fr)
            load_dkx(whh0, t_whh0T, GS, fr)
            load_dkx(wih1, t_wih1T, GS, fr)
            load_dkx(whh1, t_whh1T, GS, fr)

            b0 = wp.tile([P, GS], fp)
            nc.sync.dma_start(b0[:], t_b0[:])
            b1 = wp.tile([B, GS], fp)
            nc.sync.dma_start(b1[:], t_b1[:])

            hAT0 = wp.tile([P, KO, B], fr)
            hBT0 = wp.tile([P, KO, B], fr)
            load_dkx(hAT0, t_h0T[0], B, fr)
            load_dkx(hBT0, t_h0T[1], B, fr)

            cA = sp.tile([B, DS], fp, tag="cA")
            cB = sp.tile([B, DS], fp, tag="cB")
            nc.sync.dma_start(cA[:], t_c0s[0])
            nc.sync.dma_start(cB[:], t_c0s[1])

            x1T = wp.tile([P, KO, TB], fr)     # layer1 outputs ^T, own batches

            xg_dram = dp1.tile([T * B, GS], fp)
            a2a_in = dp1.tile([NC, T, BS, DS], fp)
            a2a_out = dp1.tile([NC, T, BS, DS], fp)

            # ---------- helpers ----------
            def emit_xg_chunk(j):
                # steps 2j, 2j+1 -> xg rows [128j, 128(j+1))
                idxt = xp.tile([P, 1], it, tag="idx")
                nc.sync.dma_start(idxt[:], t_idx[j * P:(j + 1) * P, :])
                xx = xp.tile([P, D], fp, tag="xx")
                nc.gpsimd.indirect_dma_start(
                    out=xx[:],
                    out_offset=None,
                    in_=t_emb[:],
                    in_offset=bass.IndirectOffsetOnAxis(ap=idxt[:, :1], axis=0),
                )
                xT = xp.tile([P, KO, P], fr, tag="xT")
                for k in range(KO):
                    pst = psT.tile([P, P], fp, tag="tp")
                    nc.tensor.transpose(pst[:], xx[:, k * P:(k + 1) * P], ident[:])
                    nc.vector.tensor_copy(xT[:, k, :], pst[:])
                psx = psB.tile([P, GS], fp, tag="mmB")
                for k in range(KO):
                    nc.tensor.matmul(
                        psx[:], lhsT=R(xT[:, k, :]), rhs=R(wih0[:, k, :]),
                        start=(k == 0), stop=(k == KO - 1),
                    )
                xgc = xp.tile([P, GS], fp, tag="xgc")
                nc.vector.tensor_add(xgc[:], psx[:], b0[:])
                nc.sync.dma_start(xg_dram[j * P:(j + 1) * P, :], xgc[:])

            def lstm_tail(g_sb, c_old, c_tag):
                # g_sb: (B, GS) pre-activation gates [i f o g]; returns (c_new, h)
                sig = hp.tile([B, 3 * DS], fp, tag="sig" + c_tag)
                nc.scalar.activation(sig[:], g_sb[:, 0:3 * DS], AF.Sigmoid)
                tg = hp.tile([B, DS], fp, tag="tg" + c_tag)
                nc.scalar.activation(tg[:], g_sb[:, 3 * DS:4 * DS], AF.Tanh)
                t1 = hp.tile([B, DS], fp, tag="t1" + c_tag)
                nc.vector.tensor_mul(t1[:], sig[:, DS:2 * DS], c_old[:])
                t2 = hp.tile([B, DS], fp, tag="t2" + c_tag)
                nc.vector.tensor_mul(t2[:], sig[:, 0:DS], tg[:])
                c_new = sp.tile([B, DS], fp, tag=c_tag)
                nc.vector.tensor_add(c_new[:], t1[:], t2[:])
                th = hp.tile([B, DS], fp, tag="th" + c_tag)
                nc.scalar.activation(th[:], c_new[:], AF.Tanh)
                h = hp.tile([B, DS], fp, tag="h" + c_tag)
                nc.vector.tensor_mul(h[:], sig[:, 2 * DS:3 * DS], th[:])
                # transpose h shard -> (DS, B)
                pst = psT.tile([P, P], fp, tag="tp")
                nc.tensor.transpose(pst[:DS, :B], h[:], ident[:B, :B])
                hT = hp.tile([DS, B], fr, tag="hT" + c_tag)
                nc.vector.tensor_copy(hT[:], pst[:DS, :B])
                return c_new, h, hT

            # ---------- LSTM supersteps ----------
            emit_xg_chunk(0)
            emit_xg_chunk(1)
            emit_xg_chunk(2)

            hAT_cur, hBT_cur = hAT0, hBT0

            for s in range(T + 1):
                # spread xg chunks across supersteps (PE warmth during AG
                # waits): chunk j consumed at superstep 2j, emitted at 2j-6
                if s % 2 == 0 and s // 2 + 3 <= NCH - 1:
                    emit_xg_chunk(s // 2 + 3)

                hAT_sh = None
                hBT_sh = None

                if s < T:
                    # layer0 step s: gates = xg[s] + hA_{s-1} @ whh0^T
                    ps0 = psA.tile([B, GS], fp, tag="mmA")
                    for k in range(KO):
                        nc.tensor.matmul(
                            ps0[:], lhsT=R(hAT_cur[:, k, :]), rhs=R(whh0[:, k, :]),
                            start=(k == 0), stop=(k == KO - 1),
                        )
                    xgt = hp.tile([B, GS], fp, tag="xgt")
                    nc.sync.dma_start(xgt[:], xg_dram[s * B:(s + 1) * B, :])
                    gA = hp.tile([B, GS], fp, tag="gA")
                    nc.vector.tensor_add(gA[:], ps0[:], xgt[:])
                    cA, hA, hAT_sh = lstm_tail(gA, cA, "cA")
                    if s == T - 1:
                        nc.sync.dma_start(t_cns[0], cA[:])
                        nc.sync.dma_start(t_hnT[0], hAT_sh[:].bitcast(fp))

                if s >= 1:
                    # layer1 step s-1: gates = hA_{s-1} @ wih1^T + hB_{s-2} @ whh1^T + b1
                    ps1 = psA.tile([B, GS], fp, tag="mmA")
                    for k in range(KO):
                        nc.tensor.matmul(
                            ps1[:], lhsT=R(hAT_cur[:, k, :]), rhs=R(wih1[:, k, :]),
                            start=(k == 0), stop=False,
                        )
                    for k in range(KO):
                        nc.tensor.matmul(
                            ps1[:], lhsT=R(hBT_cur[:, k, :]), rhs=R(whh1[:, k, :]),
                            start=False, stop=(k == KO - 1),
                        )
                    gB = hp.tile([B, GS], fp, tag="gB")
                    nc.vector.tensor_add(gB[:], ps1[:], b1[:])
                    cB, hB, hBT_sh = lstm_tail(gB, cB, "cB")
                    # stage layer1 output (natural layout) for the final AllToAll
                    # (one DMA per dest rank: splitting the SBUF partition dim
                    # across two AP dims mis-lowers and drops the inner dim)
                    for r in range(NC):
                        nc.sync.dma_start(
                            a2a_in[r, s - 1, :, :], hB[r * BS:(r + 1) * BS, :]
                        )
                    if s == T:
                        nc.sync.dma_start(t_cns[1], cB[:])
                        nc.sync.dma_start(t_hnT[1], hBT_sh[:].bitcast(fp))

                if s < T:
                    agi = dp.tile([2 * DS, B], fp, tag="agi")
                    nc.sync.dma_start(agi[0:DS, :], hAT_sh[:].bitcast(fp))
                    nc.sync.dma_start(
                        agi[DS:2 * DS, :],
                        (hBT_sh if hBT_sh is not None else hAT_sh)[:].bitcast(fp),
                    )
                    ago = dp.tile([2 * DS * NC, B], fp, tag="ago")
                    nc.gpsimd.collective_compute(
                        "AllGather", Alu.bypass, replica_groups=RG,
                        ins=[agi.opt()], outs=[ago.opt()],
                    )
                    hAT_new = hp.tile([P, KO, B], fr, tag="hATf")
                    hBT_new = (
                        hp.tile([P, KO, B], fr, tag="hBTf", name="hBT_new")
                        if s >= 1 else None
                    )
                    for r in range(NC):
                        nc.sync.dma_start(
                            hAT_new[:, r, :],
                            ago[2 * DS * r:2 * DS * r + DS, :].bitcast(fr),
                        )
                        if s >= 1:
                            nc.sync.dma_start(
                                hBT_new[:, r, :],
                                ago[2 * DS * r + DS:2 * DS * (r + 1), :].bitcast(fr),
                            )
                    hAT_cur = hAT_new
                    if s >= 1:
                        hBT_cur = hBT_new

            # ---------- redistribute layer1 outputs: d-shards -> batch-shards ----
            nc.gpsimd.collective_compute(
                "AllToAll", Alu.bypass, replica_groups=RG,
                ins=[a2a_in.opt()], outs=[a2a_out.opt()],
            )
            for src in range(NC):
                blk = a2a_out[src].rearrange("t j dl -> (t j) dl")
                for h2 in range(TB // P):
                    ld = ap_.tile([P, DS], fp, tag="x1ld")
                    nc.sync.dma_start(ld[:], blk[h2 * P:(h2 + 1) * P, :])
                    pst = psT.tile([P, P], fp, tag="tp")
                    nc.tensor.transpose(pst[:, :P], ld[:], ident[:])
                    nc.vector.tensor_copy(x1T[:, src, h2 * P:(h2 + 1) * P], pst[:, :P])

            # ---------- attention (own 8 batches) ----------
            # q^T = w_in^T.T @ x1T : (D_e x TB)
            qT = wp.tile([P, KO, TB], fr)
            for m in range(KO):
                wim = ap_.tile([P, KO, P], fr, tag="wim")
                nc.sync.dma_start(
                    wim[:],
                    t_winT[:, m * P:(m + 1) * P]
                    .rearrange("(ko ki) e -> ki ko e", ki=P).bitcast(fr),
                )
                psq = psB.tile([P, GS], fp, tag="mmB")
                for k in range(KO):
                    nc.tensor.matmul(
                        psq[:, :TB], lhsT=R(wim[:, k, :]), rhs=R(x1T[:, k, :]),
                        start=(k == 0), stop=(k == KO - 1),
                    )
                nc.vector.tensor_copy(qT[:, m, :], psq[:, :TB])

            qTv = qT[:].rearrange("p ko (t j) -> p ko t j", j=BS)
            wT = wp.tile([P, KO, TB], fr)   # weighted^T (cat rows 0:D)
            wTv = wT[:].rearrange("p m (t j) -> p m t j", j=BS)

            for jb in range(BS):
                ctxTb = ap_.tile([P, KO, S], fr, tag="ctxTb")
                nc.sync.dma_start(
                    ctxTb[:],
                    t_ctxT[jb].rearrange("(ko ki) s -> ki ko s", ki=P).bitcast(fr),
                )
                ctxNb = ap_.tile([S, D], fr, tag="ctxNb")
                nc.sync.dma_start(ctxNb[:], t_ctxN[jb].bitcast(fr))

                pssc = psT.tile([P, P], fp, tag="tp")
                for k in range(KO):
                    nc.tensor.matmul(
                        pssc[:T, :S], lhsT=R(qTv[:, k, :, jb]), rhs=R(ctxTb[:, k, :]),
                        start=(k == 0), stop=(k == KO - 1),
                    )
                negmax = ap_.tile([T, 1], fp, tag="negmax")
                nc.vector.tensor_reduce(
                    negmax[:], pssc[:T, :S], axis=mybir.AxisListType.X,
                    op=Alu.max, negate=True,
                )
                attn = ap_.tile([T, S], fp, tag="attn")
                sumex = ap_.tile([T, 1], fp, tag="sumex")
                nc.scalar.activation(
                    attn[:], pssc[:T, :S], AF.Exp,
                    bias=negmax[:], scale=1.0, accum_out=sumex[:],
                )
                rsum = ap_.tile([T, 1], fp, tag="rsum")
                nc.vector.reciprocal(rsum[:], sumex[:])
                nc.vector.tensor_scalar_mul(attn[:], attn[:], rsum[:])
                nc.sync.dma_start(t_attn[jb:jb + 1, :], attn[T - 1:T, :])
                if debug and jb == 0:
                    nc.sync.dma_start(t_dattn[:], attn[:])

                # attn^T (S, T)
                psat = psT.tile([P, P], fp, tag="tp")
                nc.tensor.transpose(psat[:S, :T], attn[:], ident[:T, :T])
                attnT = ap_.tile([S, T], fr, tag="attnT")
                nc.vector.tensor_copy(attnT[:], psat[:S, :T])

                # weighted^T[d, t] = sum_s ctxN[s, d] * attnT[s, t]
                for m in range(KO):
                    psw = psT.tile([P, P], fp, tag="tp")
                    nc.tensor.matmul(
                        psw[:P, :T], lhsT=R(ctxNb[:, m * P:(m + 1) * P]), rhs=R(attnT[:]),
                        start=True, stop=True,
                    )
                    nc.vector.tensor_copy(wTv[:, m, :, jb], psw[:P, :T])

            if debug:
                nc.sync.dma_start(t_dx1T[:], x1T[:].bitcast(fp))
                nc.sync.dma_start(t_dqT[:], qT[:].bitcast(fp))
                nc.sync.dma_start(t_dwT[:], wT[:].bitcast(fp))

            # out = tanh(cat @ w_out^T); cat^T k-tiles: 0..7 -> wT, 8..15 -> x1T
            outv = t_out.rearrange("t j d -> (t j) d")
            pso = [
                psA.tile([P, GS], fp, tag="mmA", name="pso0"),
                psA.tile([P, GS], fp, tag="mmA", name="pso1"),
                psB.tile([P, GS], fp, tag="mmB", name="pso2"),
                psB.tile([P, GS], fp, tag="mmB", name="pso3"),
            ]
            wo_t = []
            for k2 in range(2 * KO):
                wok = ap_.tile([P, D], fr, tag="wok")
                nc.sync.dma_start(
                    wok[:], t_woutT[k2 * P:(k2 + 1) * P, :].bitcast(fr))
                for m in range(TB // P):
                    lh = (wT if k2 < KO else x1T)[:, k2 % KO, m * P:(m + 1) * P]
                    for n2 in range(2):
                        nc.tensor.matmul(
                            pso[2 * m + n2][:, :GS],
                            lhsT=R(lh),
                            rhs=R(wok[:, n2 * GS:(n2 + 1) * GS]),
                            start=(k2 == 0), stop=(k2 == 2 * KO - 1),
                        )
            for m in range(TB // P):
                for n2 in range(2):
                    osb = ap_.tile([P, GS], fp, tag="osb")
                    nc.scalar.activation(osb[:], pso[2 * m + n2][:, :GS], AF.Tanh)
                    nc.sync.dma_start(
                        outv[m * P:(m + 1) * P, n2 * GS:(n2 + 1) * GS], osb[:]
                    )

    nc.compile()
    return nc


def _prep_inputs(inputs):
    inp = np.asarray(inputs["input"])
    h0 = np.asarray(inputs["h0"], np.float32)
    c0 = np.asarray(inputs["c0"], np.float32)
    context = np.asarray(inputs["context"], np.float32)
    emb = np.array(inputs["emb_table"], np.float32)
    emb[0] = 0.0  # padding_idx=0
    w_in = np.asarray(inputs["w_in"], np.float32)
    w_out = np.asarray(inputs["w_out"], np.float32)

    idx = inp.astype(np.int32).reshape(T * B, 1)
    h0T = np.ascontiguousarray(h0.transpose(0, 2, 1))
    w_inT = np.ascontiguousarray(w_in.T)
    w_outT = np.ascontiguousarray(w_out.T)

    maps = []
    for c in range(NC):
        rows = _gate_rows(c)
        m = {
            "idx": idx,
            "emb": emb,
            "h0T": h0T,
            "c0s": np.ascontiguousarray(c0[:, :, c * DS:(c + 1) * DS]),
            "wih0T": np.ascontiguousarray(np.asarray(inputs["wih0"], np.float32)[rows].T),
            "whh0T": np.ascontiguousarray(np.asarray(inputs["whh0"], np.float32)[rows].T),
            "wih1T": np.ascontiguousarray(np.asarray(inputs["wih1"], np.float32)[rows].T),
            "whh1T": np.ascontiguousarray(np.asarray(inputs["whh1"], np.float32)[rows].T),
            "b0bc": np.ascontiguousarray(np.broadcast_to(
                (np.asarray(inputs["bih0"], np.float32)
                 + np.asarray(inputs["bhh0"], np.float32))[rows], (P, GS))),
            "b1bc": np.ascontiguousarray(np.broadcast_to(
                (np.asarray(inputs["bih1"], np.float32)
                 + np.asarray(inputs["bhh1"], np.float32))[rows], (B, GS))),
            "winT": w_inT,
            "woutT": w_outT,
            "ctxT": np.ascontiguousarray(
                context[:, c * BS:(c + 1) * BS, :].transpose(1, 2, 0)),
            "ctxN": np.ascontiguousarray(
                context[:, c * BS:(c + 1) * BS, :].transpose(1, 0, 2)),
        }
        maps.append(m)
    return maps


def kernel(**inputs):
    from concourse import bass_utils

    if "nc" not in _CACHE:
        _CACHE["nc"] = _build()
    nc = _CACHE["nc"]

    in_maps = _prep_inputs(inputs)
    trace = os.environ.get("KBENCH_TRACE", "0") == "1"
    res = bass_utils.run_bass_kernel_spmd(
        nc, in_maps, core_ids=list(range(NC)), trace=trace,
    )
    _CACHE["last_result"] = res

    out = np.concatenate([r["out_s"] for r in res.results], axis=1)
    hn = np.concatenate(
        [r["hnT_s"].transpose(0, 2, 1) for r in res.results], axis=2)
    cn = np.concatenate([r["cn_s"] for r in res.results], axis=2)
    attn_last = np.concatenate([r["attn_s"] for r in res.results], axis=0)
    return out, (hn, cn), attn_last
